# revision 15
# baseline (speedup 1.0000x reference)
"""GCLayer (graph-conv) kernel for Trainium2, 8-core data-parallel.

Math (see reference): out[b,c,h,w] = conv3x3_zeropad(sum_ci x[b,ci])[c,h,w]
                                     + CIN * bias[c]

Per-core plan (4 batches/core, processed as 2 pairs of batches):
  1. channel-sum on PE: x laid out [partition=(b2,ci64)=128, free=spatial],
     matmul with block-ones lhsT [128,2] -> PSUM [2,512] chunks.
  2. evacuate PSUM -> rows {0,8} of xs_rep[16, 132*132]: a zero-padded
     image plane per batch (DVE/ACT alternating, strided dest).
  3. two SBUF->SBUF DMAs replicate each plane into 8 shift-baked rows:
     row (b*8 + dh2*4 + dw) = plane shifted by dh2*132 + dw.
  4. conv as 2 accumulating matmuls per 512-chunk: lhsT kmatA/kmatB
     [16,128] (block-diag taps; dead shift rows zeroed) x strided rhs
     views of xs_rep -> PSUM [128=(b2,c64), 512].  The dh=2 taps come
     from reading the dh2=0 rows at +2 rows of free offset.
  5. evacuate + bias (ACT/DVE) -> SBUF out tiles -> 2MB DMAs to DRAM.
"""

import os

import numpy as np

import concourse.bass as bass
import concourse.mybir as mybir
from concourse import bacc
from concourse.bass import AP
from concourse.bass_utils import run_bass_kernel_spmd
from concourse.tile import TileContext

N_CORES = 8
B, CIN, COUT, IMG = 32, 64, 64, 128
BPC = B // N_CORES          # batches per core = 4
NPAIR = BPC // 2            # pairs per core = 2
HW = IMG * IMG              # 16384
PITCH = IMG + 4             # 132 padded row pitch
PADSZ = PITCH * PITCH       # 17424 (132 rows x 132 cols)
XF = 4096                   # x/out tile free size (2 MB DMAs)
CH = 512                    # matmul free-dim chunk
NCHUNK = HW // CH           # 32 chunks/pair; chunk c = image rows 4c..4c+3
RLEN = PADSZ - PITCH - 3    # replication copy length (max shift 132+3)

F32 = mybir.dt.float32

_cache = {}

LAST_RESULTS = None


def _build_nc():
    # Bacc (not raw Bass): its compile pipeline legalizes sync waits --
    # TRN2 instructions support at most 1 wait, Bacc splits the rest
    # into event-semaphore instructions.
    nc = bacc.Bacc("TRN2", num_devices=N_CORES, debug=False)

    xin = nc.dram_tensor("xin", [BPC, CIN, IMG, IMG], F32, kind="ExternalInput")
    kmat_d = nc.dram_tensor("kmat", [2, 16, 128], F32, kind="ExternalInput")
    ones_d = nc.dram_tensor("ones2", [128, 2], F32, kind="ExternalInput")
    bias_d = nc.dram_tensor("biasc", [128, 1], F32, kind="ExternalInput")
    out_d = nc.dram_tensor("out", [BPC, CIN, IMG, IMG], F32, kind="ExternalOutput")

    x_flat = xin[:].rearrange("b c h w -> (b c) (h w)")    # [256, 16384]
    o_flat = out_d[:].rearrange("b c h w -> (b c) (h w)")  # [256, 16384]

    with TileContext(nc) as tc:
        with (
            tc.tile_pool(name="const", bufs=1) as cpool,
            tc.tile_pool(name="x", bufs=3) as xpool,
            tc.tile_pool(name="o", bufs=3) as opool,
            tc.tile_pool(name="xs", bufs=1) as xspool,
            tc.tile_pool(name="ps_sum", bufs=3, space="PSUM") as psum_s,
            tc.tile_pool(name="ps_conv", bufs=4, space="PSUM") as psum_c,
        ):
            ones_t = cpool.tile([128, 2], F32)
            nc.sync.dma_start(ones_t[:], ones_d[:])
            kA_t = cpool.tile([16, 128], F32)
            nc.sync.dma_start(kA_t[:], kmat_d[0])
            kB_t = cpool.tile([16, 128], F32)
            nc.sync.dma_start(kB_t[:], kmat_d[1])
            bias_t = cpool.tile([128, 1], F32)
            nc.sync.dma_start(bias_t[:], bias_d[:])

            # 16 shift-baked replica rows of the padded channel-sum planes.
            # Row map (kmat rows are permuted to match, host-side):
            #   q=0: (b0,dh2=0,dw=0) master   q=1: (b1,0,0) master
            #   q=2..4:  (b0,0,dw=1..3)       q=5..8:  (b0,1,dw=0..3)
            #   q=9..11: (b1,0,dw=1..3)       q=12..15:(b1,1,dw=0..3)
            xr = xspool.tile([16, PADSZ], F32)
            m = xr[0:2]                         # [2, PADSZ] master rows
            mv = m.rearrange("p (h w) -> p h w", w=PITCH)   # [2,132,132]
            # zero the pad cells of the master planes (replicas inherit)
            nc.gpsimd.memset(m[:, 0:PITCH], 0.0)                  # top row
            nc.gpsimd.memset(mv[:, 1:131, 0:1], 0.0)              # left col
            nc.gpsimd.memset(mv[:, 1:129, IMG + 1 : PITCH], 0.0)  # right cols
            nc.gpsimd.memset(m[:, (IMG + 1) * PITCH :], 0.0)      # bottom rows

            xrv = xr[:].rearrange("p (h w) -> p h w", w=PITCH)  # [16,132,132]
            x_off = xr[:].offset

            for p in range(NPAIR):
                rows = slice(p * 128, (p + 1) * 128)
                # ---- phase A: load x, channel-sum, evacuate to master rows
                for xt_i in range(HW // XF):  # 4 tiles
                    xt = xpool.tile([128, XF], F32)
                    nc.sync.dma_start(
                        xt[:], x_flat[rows, xt_i * XF : (xt_i + 1) * XF]
                    )
                    for j in range(XF // CH):  # 8 chunks
                        c = xt_i * (XF // CH) + j  # global chunk 0..31
                        ps = psum_s.tile([2, CH], F32)
                        nc.tensor.matmul(
                            ps[:],
                            ones_t[:],
                            xt[:, j * CH : (j + 1) * CH],
                            start=True,
                            stop=True,
                        )
                        # chunk c -> padded rows 4c+1..4c+4, cols 1..128
                        dest = mv[:, 1 + 4 * c : 5 + 4 * c, 1 : 1 + IMG]
                        src = ps[:].rearrange("p (h w) -> p h w", w=IMG)
                        if c % 2 == 0:
                            nc.scalar.copy(dest, src)
                        else:
                            nc.vector.tensor_copy(out=dest, in_=src)

                # ---- replicate masters into shift-baked rows.
                # SBUF AP rule: only dim0 steps partitions; other dims are
                # offsets within a partition.  So each DMA writes one
                # contiguous partition run, reading one master partition
                # with the shift in the free offset.  DMA last dim must
                # stay < 16K f32 -> 2 length pieces.
                S1 = 64 * PITCH
                for lo, ln in ((0, S1), (S1, RLEN - S1)):
                    for b in range(2):
                        # R1: rows 2..4 (+7b) = master_b shifted by dw=1..3
                        nc.sync.dma_start(
                            AP(xr.tensor, x_off + (2 + 7 * b) * PADSZ + lo,
                               [[PADSZ, 3], [1, ln]]),
                            AP(xr.tensor, x_off + b * PADSZ + 1 + lo,
                               [[PADSZ, 1], [1, 3], [1, ln]]),
                        )
                        # R2: rows 5..8 (+7b) = master_b shifted by 132+dw
                        nc.sync.dma_start(
                            AP(xr.tensor, x_off + (5 + 7 * b) * PADSZ + lo,
                               [[PADSZ, 4], [1, ln]]),
                            AP(xr.tensor, x_off + b * PADSZ + PITCH + lo,
                               [[PADSZ, 1], [1, 4], [1, ln]]),
                        )

                # ---- phase B: conv (2 accumulating matmuls per chunk) + bias
                for ot_i in range(HW // XF):  # 4 out tiles
                    ot = opool.tile([128, XF], F32)
                    for j in range(XF // CH):  # 8 chunks
                        c = ot_i * (XF // CH) + j
                        pc = psum_c.tile([128, CH], F32)
                        # taps dh=0,1 from the dh2-baked rows at +0
                        nc.tensor.matmul(
                            pc[:],
                            kA_t[:],
                            xrv[:, 4 * c : 4 * c + 4, 0:IMG],
                            start=True,
                            stop=False,
                        )
                        # taps dh=2 from the dh2=0 rows at +2 rows
                        nc.tensor.matmul(
                            pc[:],
                            kB_t[:],
                            xrv[:, 4 * c + 2 : 4 * c + 6, 0:IMG],
                            start=False,
                            stop=True,
                        )
                        dst = ot[:, j * CH : (j + 1) * CH]
                        if c % 2 == 0:
                            nc.vector.tensor_scalar_add(dst, pc[:], bias_t[:])
                        else:
                            nc.scalar.add(dst, pc[:], bias_t[:])
                    nc.sync.dma_start(
                        o_flat[rows, ot_i * XF : (ot_i + 1) * XF], ot[:]
                    )
    nc.finalize()
    return nc


def kernel(**inputs):
    global LAST_RESULTS
    x = np.ascontiguousarray(np.asarray(inputs["x"], dtype=np.float32))
    kern = np.asarray(inputs["kernel"], dtype=np.float32)
    bias = np.asarray(inputs["bias"], dtype=np.float32)

    if "nc" not in _cache:
        _cache["nc"] = _build_nc()
    nc = _cache["nc"]

    ones2 = np.zeros((128, 2), np.float32)
    ones2[0:64, 0] = 1.0
    ones2[64:128, 1] = 1.0

    # row map q -> (b, dh2, dw); must match the device-side layout
    rowmap = [(0, 0, 0), (1, 0, 0)]
    rowmap += [(0, 0, dw) for dw in (1, 2, 3)]
    rowmap += [(0, 1, dw) for dw in (0, 1, 2, 3)]
    rowmap += [(1, 0, dw) for dw in (1, 2, 3)]
    rowmap += [(1, 1, dw) for dw in (0, 1, 2, 3)]
    # kmat[0] = kmatA: taps (dh=dh2, dw); kmat[1] = kmatB: taps (dh=2, dw)
    # from the dh2=0 rows.  dw=3 rows are dead (zero weights).
    kmat = np.zeros((2, 16, 128), np.float32)
    for q, (b, dh2, dw) in enumerate(rowmap):
        if dw < 3:
            kmat[0, q, b * 64 : (b + 1) * 64] = kern[:, dh2, dw]
            if dh2 == 0:
                kmat[1, q, b * 64 : (b + 1) * 64] = kern[:, 2, dw]

    biasc = (np.tile(bias.reshape(COUT), 2).reshape(128, 1) * float(CIN)).astype(
        np.float32
    )

    in_maps = []
    for core in range(N_CORES):
        in_maps.append(
            {
                "xin": np.ascontiguousarray(x[core * BPC : (core + 1) * BPC]),
                "kmat": kmat,
                "ones2": ones2,
                "biasc": biasc,
            }
        )

    res = run_bass_kernel_spmd(
        nc,
        in_maps,
        core_ids=list(range(N_CORES)),
        trace=bool(os.environ.get("KERNEL_TRACE")),
    )
    LAST_RESULTS = res
    out = np.concatenate([r["out"] for r in res.results], axis=0)
    return out
